# revision 11
# baseline (speedup 1.0000x reference)
"""Multi-head attention with sparse mask + post-softmax group_prob modulation.

B=8, S=1024, D=1024, H=16 heads (DK=64). Sharded batch-parallel across 8
NeuronCores (one batch element per core). Inputs are marshaled on host into
feature-major (transposed) layouts so every matmul contraction runs with the
contract dim on SBUF partitions:

  per core (batch b):
    qT/kT/vT = x[b].T          [D, S]   fp32
    maskT    = mask[b].T       [S, S]   int32   (k-major)
    gT       = group_prob[b].T [S, S]   fp32    (k-major)

  device pipeline:
    V        = vT-chunk-stationary x Wv moving  -> [s, dv] bf16 (+bv via DVE)
    A.T      = (maskT==0)*-1e9, diag forced 0 (bf16)
    per head pair j (heads 2j, 2j+1 in partition halves of dk-chunk j):
      KT/QT[j] = Wq/Wk-chunk-stationary matmuls -> [dk, s] fp32 (+bias, ACT)
      scores.T[k,q] psum = KT_h^T @ QT_h (fp32r, K=64, 2-head row-packed)
                   += eye64 @ A.T        (mask add, 2 packed identity MMs)
      e_m = exp(psum / sqrt(dk)) -> bf16 (ACT reads PSUM, scale folds 1/8)
      denom[q] = ones^T @ e_m (PE partition-reduce, accum over k chunks)
      e_g = e_m * gT (DVE bf16)
      x.T psum[dv,q] = V-slice-stationary @ e_g (col-packed head pair)
      X.T = psum * R -> DRAM scratch (R = recip rows broadcast by K=1 MM)
    out[q,:] = X.T-chunk-stationary @ Wo + bo -> DMA out (natural layout)
"""

import os
from contextlib import ExitStack

import ml_dtypes
import numpy as np

import concourse.bacc as bacc
import concourse.bass as bass
import concourse.mybir as mybir
import concourse.tile as tile

B, S, D, H = 8, 1024, 1024, 16
DK = D // H  # 64
NCH = S // 128  # 8 chunks of 128
NEG = -1.0e9
F32 = mybir.dt.float32
F32R = mybir.dt.float32r
BF16 = mybir.dt.bfloat16
I32 = mybir.dt.int32
AF = mybir.ActivationFunctionType
ALU = mybir.AluOpType

_CACHE = {}


def r(ap):
    """view fp32 AP as float32r for full-rate matmul"""
    return ap.bitcast(F32R)


def emit_kernel(ctx: ExitStack, tc: tile.TileContext, io: dict):
    nc = tc.nc
    qT, kT, vT = io["qT"], io["kT"], io["vT"]
    maskT, gT = io["maskT"], io["gT"]
    Wq, Wk, Wv, Wo = io["Wq"], io["Wk"], io["Wv"], io["Wo"]
    bq, bk = io["bq"], io["bk"]
    BV, BO = io["BV"], io["BO"]
    eyeb, inveyeb = io["eyeb"], io["inveyeb"]
    out = io["out"]

    # ---------------- pools ----------------
    res = ctx.enter_context(tc.tile_pool(name="res", bufs=1))
    instream = ctx.enter_context(tc.tile_pool(name="instream", bufs=2))
    qkw = ctx.enter_context(tc.tile_pool(name="qkw", bufs=2))
    qkt = ctx.enter_context(tc.tile_pool(name="qkt", bufs=2))
    em_pool = ctx.enter_context(tc.tile_pool(name="em", bufs=3))
    eg_pool = ctx.enter_context(tc.tile_pool(name="eg", bufs=3))
    small = ctx.enter_context(tc.tile_pool(name="small", bufs=1))
    outp = ctx.enter_context(tc.tile_pool(name="outp", bufs=2))
    dram = ctx.enter_context(tc.tile_pool(name="dram", bufs=1, space="DRAM"))

    psum_proj = ctx.enter_context(tc.tile_pool(name="ps_proj", bufs=2, space="PSUM"))
    psum_s = ctx.enter_context(tc.tile_pool(name="ps_s", bufs=2, space="PSUM"))
    psum_x = ctx.enter_context(tc.tile_pool(name="ps_x", bufs=2, space="PSUM"))
    psum_d = ctx.enter_context(tc.tile_pool(name="ps_d", bufs=2, space="PSUM"))

    # ---------------- constants ----------------
    ones_col = small.tile([128, 1], BF16)  # denominator stationary
    nc.gpsimd.memset(ones_col[:], 1.0)
    ones_row = small.tile([1, 64], F32)  # recip broadcast stationary (K=1)
    nc.gpsimd.memset(ones_row[:], 1.0)
    eye_sb = small.tile([128, 128], BF16)
    nc.sync.dma_start(eye_sb[:], eyeb[:, :])
    inveye_sb = small.tile([128, 128], BF16)
    nc.sync.dma_start(inveye_sb[:], inveyeb[:, :])
    BV_sb = small.tile([128, D], F32)
    nc.sync.dma_start(BV_sb[:], BV[:, :])
    BO_sb = small.tile([128, D], F32)
    nc.sync.dma_start(BO_sb[:], BO[:, :])
    bq_sb = small.tile([128, NCH], F32)  # column ck = bias chunk ck
    bk_sb = small.tile([128, NCH], F32)
    for ck in range(NCH):
        nc.sync.dma_start(bq_sb[:, ck : ck + 1], bq[ck * 128 : (ck + 1) * 128, 0:1])
        nc.sync.dma_start(bk_sb[:, ck : ck + 1], bk[ck * 128 : (ck + 1) * 128, 0:1])

    qscale = 1.0 / float(np.sqrt(DK))  # folded into exp's activation scale

    # ---------------- V = value @ Wv + bv  -> bf16, natural [s, dv] ---------
    # vT and Wv transiently resident (freed after this phase).
    V_sb = res.tile([128, NCH * D], BF16)  # col block kc -> V[128*kc:+128, :]
    vT_res = res.tile([128, NCH * S], F32R, tag="big_a")  # vT[128c:+128, :]
    Wv_res = res.tile([128, NCH * D], F32R, tag="big_b")  # Wv[128c:+128, :]
    for c in range(NCH):
        nc.sync.dma_start(vT_res[:, c * S : (c + 1) * S], vT[c * 128 : (c + 1) * 128, :])
        nc.sync.dma_start(Wv_res[:, c * D : (c + 1) * D], Wv[c * 128 : (c + 1) * 128, :])
    for kc in range(NCH):
        for dt in range(2):
            ps = psum_proj.tile([128, 512], F32)
            for c in range(NCH):
                nc.tensor.matmul(
                    ps[:],
                    r(vT_res[:, c * S + kc * 128 : c * S + kc * 128 + 128]),
                    r(Wv_res[:, c * D + dt * 512 : c * D + (dt + 1) * 512]),
                    start=(c == 0),
                    stop=(c == NCH - 1),
                )
            nc.vector.tensor_add(
                V_sb[:, kc * D + dt * 512 : kc * D + (dt + 1) * 512],
                ps[:],
                BV_sb[:, dt * 512 : (dt + 1) * 512],
            )

    # ---------------- A.T (additive mask, bf16) + gT cast -------------------
    A_sb = res.tile([128, NCH * S], BF16)  # col block kc -> A.T[128kc:+128, :]
    G_sb = res.tile([128, NCH * S], BF16)
    for kc in range(NCH):
        m_t = instream.tile([128, S], I32, tag="mg")
        nc.sync.dma_start(m_t[:], maskT[kc * 128 : (kc + 1) * 128, :])
        nc.vector.tensor_scalar(
            A_sb[:, kc * S : (kc + 1) * S], m_t[:], 0, NEG, ALU.is_equal, ALU.mult
        )
        g_t = instream.tile([128, S], F32, tag="mg")
        nc.sync.dma_start(g_t[:], gT[kc * 128 : (kc + 1) * 128, :])
        nc.vector.tensor_copy(G_sb[:, kc * S : (kc + 1) * S], g_t[:])
        # zero the diagonal block entries (always-allowed positions)
        blk = slice(kc * S + kc * 128, kc * S + kc * 128 + 128)
        nc.vector.tensor_mul(A_sb[:, blk], A_sb[:, blk], inveye_sb[:])

    # ---------------- resident qT / kT --------------------------------------
    qT_res = res.tile([128, NCH * S], F32R, tag="big_a")  # qT[128c:+128, :]
    kT_res = res.tile([128, NCH * S], F32R, tag="big_b")
    for c in range(NCH):
        nc.sync.dma_start(qT_res[:, c * S : (c + 1) * S], qT[c * 128 : (c + 1) * 128, :])
        nc.sync.dma_start(kT_res[:, c * S : (c + 1) * S], kT[c * 128 : (c + 1) * 128, :])

    # ---------------- attention per head pair -------------------------------
    XT_d = dram.tile([D, S], F32R)  # X.T scratch: rows 128j = pair j
    recip_pool = ctx.enter_context(tc.tile_pool(name="recip", bufs=1))

    for j in range(NCH):  # head pair j = heads 2j, 2j+1
        # --- project QT[j], KT[j]: [128 dk, S] ---
        QT_t = qkt.tile([128, S], F32R, tag="QT")
        KT_t = qkt.tile([128, S], F32R, tag="KT")
        for W, src_res, dst, bias_sb in (
            (Wq, qT_res, QT_t, bq_sb),
            (Wk, kT_res, KT_t, bk_sb),
        ):
            w_t = qkw.tile([128, 128 * NCH], F32R, tag="w")
            for c in range(NCH):
                nc.sync.dma_start(
                    w_t[:, c * 128 : (c + 1) * 128],
                    W[c * 128 : (c + 1) * 128, j * 128 : (j + 1) * 128],
                )
            for qt in range(2):
                ps = psum_proj.tile([128, 512], F32)
                for c in range(NCH):
                    nc.tensor.matmul(
                        ps[:],
                        r(w_t[:, c * 128 : (c + 1) * 128]),
                        r(src_res[:, c * S + qt * 512 : c * S + (qt + 1) * 512]),
                        start=(c == 0),
                        stop=(c == NCH - 1),
                    )
                nc.scalar.activation(
                    dst[:, qt * 512 : (qt + 1) * 512],
                    ps[:],
                    AF.Identity,
                    bias=bias_sb[:, j : j + 1],
                )

        # --- scores + exp + denom + G-mul + PV ---
        ps_dd = [
            psum_d.tile([33, 512], F32, name="psdd", tag="psdd") for _ in range(2)
        ]
        ps_xx = [
            psum_x.tile([128, 512], F32, name="psxx", tag="psxx") for _ in range(2)
        ]
        for kc in range(NCH):
            eg_t = [None, None]
            for h in range(2):
                em = em_pool.tile([128, S], BF16, tag="em")
                for qt in range(2):
                    ps = psum_s.tile([128, 512], F32)
                    asl = slice(kc * S + qt * 512, kc * S + (qt + 1) * 512)
                    nc.tensor.matmul(
                        ps[0:64, :],
                        eye_sb[0:64, 0:64],
                        A_sb[0:64, asl],
                        start=True,
                        stop=False,
                        tile_position=(0, 0),
                        skip_group_check=True,
                    )
                    nc.tensor.matmul(
                        ps[64:128, :],
                        eye_sb[64:128, 64:128],
                        A_sb[64:128, asl],
                        start=True,
                        stop=False,
                        tile_position=(64, 64),
                        skip_group_check=True,
                    )
                    nc.tensor.matmul(
                        ps[:],
                        r(KT_t[64 * h : 64 * h + 64, kc * 128 : (kc + 1) * 128]),
                        r(QT_t[64 * h : 64 * h + 64, qt * 512 : (qt + 1) * 512]),
                        start=False,
                        stop=True,
                        tile_position=(64 * h, 0),
                        skip_group_check=True,
                    )
                    # e_m = exp(scores/sqrt(dk)); masked entries underflow to 0
                    nc.scalar.activation(
                        em[:, qt * 512 : (qt + 1) * 512], ps[:], AF.Exp, scale=qscale
                    )
                    # denominator: ones^T @ e_m accumulated over kc
                    nc.tensor.matmul(
                        ps_dd[qt][32 * h : 32 * h + 1, :],
                        ones_col[:, 0:1],
                        em[:, qt * 512 : (qt + 1) * 512],
                        start=(kc == 0),
                        stop=(kc == NCH - 1),
                        tile_position=(0, 32 * h),
                        skip_group_check=True,
                    )
                # e_g = e_m * G
                eg = eg_pool.tile([128, S], BF16, tag="eg")
                eg_t[h] = eg
                nc.vector.tensor_mul(eg[:], em[:], G_sb[:, kc * S : (kc + 1) * S])
            # PV: col-packed pair, accumulate over kc
            for qt in range(2):
                for h in range(2):
                    hd = 64 * (2 * j + h)
                    nc.tensor.matmul(
                        ps_xx[qt][64 * h : 64 * h + 64, :],
                        V_sb[:, kc * D + hd : kc * D + hd + 64],
                        eg_t[h][:, qt * 512 : (qt + 1) * 512],
                        start=(kc == 0),
                        stop=(kc == NCH - 1),
                        tile_position=(0, 64 * h),
                        skip_group_check=True,
                    )
        # reciprocal of denominators (rows 0 and 32 of ps_dd)
        recip_t = [
            recip_pool.tile([1, S], F32, name=f"recip{h}", tag=f"recip{h}")
            for h in range(2)
        ]
        for qt in range(2):
            for h in range(2):
                nc.vector.reciprocal(
                    recip_t[h][0:1, qt * 512 : (qt + 1) * 512],
                    ps_dd[qt][32 * h : 32 * h + 1, :],
                )
        # R = broadcast recip rows via K=1 matmul, evict, X.T = x * R -> DRAM
        xt_t = outp.tile([128, S], F32R, tag="xt")
        for qt in range(2):
            ps_r = psum_proj.tile([128, 512], F32, tag="ps")
            for h in range(2):
                nc.tensor.matmul(
                    ps_r[64 * h : 64 * h + 64, :],
                    ones_row[0:1, 0:64],
                    recip_t[h][0:1, qt * 512 : (qt + 1) * 512],
                    start=True,
                    stop=True,
                    tile_position=(0, 64 * h),
                    skip_group_check=True,
                )
            r_sb = outp.tile([128, 512], F32, tag="rsb")
            nc.scalar.activation(r_sb[:], ps_r[:], AF.Copy, bias=0.0)
            nc.vector.tensor_mul(
                xt_t[:, qt * 512 : (qt + 1) * 512], ps_xx[qt][:], r_sb[:]
            )
        nc.sync.dma_start(XT_d[j * 128 : (j + 1) * 128, :], xt_t[:])

    # ---------------- out = X @ Wo + bo -------------------------------------
    Wo_res = res.tile([128, NCH * D], F32R, tag="big_a")
    for c in range(NCH):
        nc.sync.dma_start(Wo_res[:, c * D : (c + 1) * D], Wo[c * 128 : (c + 1) * 128, :])
    for qs in range(NCH):
        xin = instream.tile([128, 128 * NCH], F32R, tag="xin")
        for c in range(NCH):
            nc.sync.dma_start(
                xin[:, c * 128 : (c + 1) * 128],
                XT_d[c * 128 : (c + 1) * 128, qs * 128 : (qs + 1) * 128],
            )
        o_sb = outp.tile([128, D], F32, tag="osb")
        for dt in range(2):
            ps = psum_proj.tile([128, 512], F32)
            for c in range(NCH):
                nc.tensor.matmul(
                    ps[:],
                    r(xin[:, c * 128 : (c + 1) * 128]),
                    r(Wo_res[:, c * D + dt * 512 : c * D + (dt + 1) * 512]),
                    start=(c == 0),
                    stop=(c == NCH - 1),
                )
            nc.vector.tensor_add(
                o_sb[:, dt * 512 : (dt + 1) * 512],
                ps[:],
                BO_sb[:, dt * 512 : (dt + 1) * 512],
            )
        nc.sync.dma_start(out[qs * 128 : (qs + 1) * 128, :], o_sb[:])


def build_module():
    if "nc" in _CACHE:
        return _CACHE["nc"], _CACHE["io"]
    nc = bacc.Bacc(
        "TRN2", target_bir_lowering=False, debug=False, enable_asserts=False
    )
    io = {}
    for name in ("qT", "kT", "vT"):
        io[name] = nc.dram_tensor(name, [S, S], F32R, kind="ExternalInput").ap()
    io["gT"] = nc.dram_tensor("gT", [S, S], F32, kind="ExternalInput").ap()
    io["maskT"] = nc.dram_tensor("maskT", [S, S], I32, kind="ExternalInput").ap()
    for name in ("Wq", "Wk", "Wv", "Wo"):
        io[name] = nc.dram_tensor(name, [D, D], F32R, kind="ExternalInput").ap()
    for name in ("bq", "bk"):
        io[name] = nc.dram_tensor(name, [D, 1], F32, kind="ExternalInput").ap()
    for name in ("BV", "BO"):
        io[name] = nc.dram_tensor(name, [128, D], F32, kind="ExternalInput").ap()
    io["eyeb"] = nc.dram_tensor("eyeb", [128, 128], BF16, kind="ExternalInput").ap()
    io["inveyeb"] = nc.dram_tensor(
        "inveyeb", [128, 128], BF16, kind="ExternalInput"
    ).ap()
    io["out"] = nc.dram_tensor("out", [S, D], F32, kind="ExternalOutput").ap()

    with tile.TileContext(nc) as tc:
        with ExitStack() as ctx:
            emit_kernel(ctx, tc, io)
    nc.compile()
    _CACHE["nc"] = nc
    _CACHE["io"] = io
    return nc, io


def make_in_maps(**inputs):
    f32 = np.float32
    shared = {
        "Wq": np.ascontiguousarray(inputs["Wq"], f32),
        "Wk": np.ascontiguousarray(inputs["Wk"], f32),
        "Wv": np.ascontiguousarray(inputs["Wv"], f32),
        "Wo": np.ascontiguousarray(inputs["Wo"], f32),
        "bq": np.ascontiguousarray(np.reshape(inputs["bq"], (D, 1)), f32),
        "bk": np.ascontiguousarray(np.reshape(inputs["bk"], (D, 1)), f32),
        "BV": np.ascontiguousarray(
            np.tile(np.reshape(inputs["bv"], (1, D)), (128, 1)), f32
        ),
        "BO": np.ascontiguousarray(
            np.tile(np.reshape(inputs["bo"], (1, D)), (128, 1)), f32
        ),
        "eyeb": np.eye(128, dtype=ml_dtypes.bfloat16),
        "inveyeb": (1.0 - np.eye(128, dtype=np.float32)).astype(ml_dtypes.bfloat16),
    }
    q, k, v = (np.asarray(inputs[n], f32) for n in ("query", "key", "value"))
    gp = np.asarray(inputs["group_prob"], f32)
    mk = np.asarray(inputs["mask"], np.int32)
    in_maps = []
    for b in range(B):
        m = dict(shared)
        m["qT"] = np.ascontiguousarray(q[b].T)
        m["kT"] = np.ascontiguousarray(k[b].T)
        m["vT"] = np.ascontiguousarray(v[b].T)
        m["gT"] = np.ascontiguousarray(gp[b].T)
        m["maskT"] = np.ascontiguousarray(mk[b].T)
        in_maps.append(m)
    return in_maps


def kernel(**inputs) -> np.ndarray:
    from concourse.bass_utils import run_bass_kernel_spmd

    nc, _ = build_module()
    in_maps = make_in_maps(**inputs)
    trace = bool(int(os.environ.get("KERNEL_TRACE", "0")))
    res = run_bass_kernel_spmd(nc, in_maps, core_ids=list(range(B)), trace=trace)
    _CACHE["last_result"] = res
    return np.stack([res.results[b]["out"] for b in range(B)], axis=0)
